# revision 34
# baseline (speedup 1.0000x reference)
"""Cross-attention (queries from x, keys/values from yz, additive met bias)
distributed over 8 TRN2 NeuronCores.

Module (B=4, N=2048, DIM=512, H=8, D=64):
    q  = split_heads(x @ Wq);  k = split_heads(yz @ Wk);  v = split_heads(yz @ Wv)
    sim  = (q k^T + q_met k_met^T) * D**-0.5          # (B*H, N, N)
    attn = softmax(sim, -1)
    out  = merge_heads(attn @ v) @ Wo + bo
    returns (out, attn)

Sharding: batch*head parallel.  Core i handles batch b=i//2, heads
[4*(i%2), 4*(i%2)+4).  Weights are sliced per head group; x/met/yz are
pre-transposed to DIM-major bf16 on host; v (+ a ones column) is host
pre-projected.  No collectives: the only cross-core reduction (the two
half-head Wo partials per batch) is a host-side add during unshard.

Device layout: everything j-major (S^T):
 * scores: S^T = K2T.T @ Q2T with the met bias packed along the K=128
   contraction ([k^T;km^T] x [q^T;qm^T]) -- the bias add is free.
 * out:    out^T = v(65-col,ones-augmented)-as-lhsT @ exp(S^T); PSUM row 64
   accumulates the softmax denominators (free row-sum).
 * attn is written transposed in bf16 (dense DMA rows) and un-transposed /
   upcast to f32 on host during unshard.

Walrus only allows ONE sync-wait on a Matmult; Tile sometimes emits 2-3
(recycled-PSUM WAR + WAW).  _legalize_mm_waits post-processes the BIR and
hoists every excess wait onto an inserted same-engine NoOp directly before
the matmul (the NX sequencer executes them in order -- semantically
identical).
"""

import os
import sys

import numpy as np

_TRN_REPO = "/opt/trn_rl_repo"
if _TRN_REPO not in sys.path:
    sys.path.insert(0, _TRN_REPO)

import ml_dtypes

BF16 = ml_dtypes.bfloat16

B = 4
N = 2048
DIM = 512
HEADS = 8
D = 64
INNER = HEADS * D
SCALE = float(D) ** -0.5  # 0.125
NCORES = 8
NH = 4  # heads per core
NJ = N // 128  # 16 j-chunks
KC = DIM // 128  # 4 contraction chunks for projections

_built = None


def _legalize_mm_waits(nc):
    """Walrus codegen accepts at most one sync-wait per Matmult.  Move any
    extra waits onto same-engine NoOps inserted right before the matmul."""
    import concourse.mybir as mybir

    cnt = 0
    for fn in nc.m.functions:
        for blk in fn.blocks:
            out = []
            changed = False
            for ins in blk.instructions:
                si = getattr(ins, "sync_info", None)
                hoistable = type(ins).__name__ in {
                    "InstMatmult",
                    "InstMatmultMx",
                    "InstActivation",
                    "InstLdweights",
                    "InstDMA",
                    "InstDMACopy",
                    "InstDmaTransposeAnt",
                    "InstTensorCopy",
                    "InstTensorTensor",
                    "InstTensorScalarPtr",
                    "InstTensorReduce",
                    "InstMemSet",
                    "InstDrain",
                    "InstReciprocal",
                    "InstCopy",
                    "InstIota",
                    "InstBNStats",
                    "InstBNStatsAggregate",
                }
                if (
                    hoistable
                    and si is not None
                    and si.on_wait
                    and len(si.on_wait) > 1
                ):
                    for wt in si.on_wait:
                        out.append(
                            mybir.InstNoOp(
                                name=f"I-mmwait-{cnt}",
                                engine=ins.engine,
                                ins=[],
                                outs=[],
                                sync_info=mybir.SyncInfo(
                                    on_wait=[wt], on_update=[]
                                ),
                            )
                        )
                        cnt += 1
                    ins.sync_info = mybir.SyncInfo(
                        on_wait=[], on_update=si.on_update
                    )
                    changed = True
                out.append(ins)
            if changed:
                blk.instructions[:] = out
    return cnt


def _build_nc():
    import concourse.bass as bass
    import concourse.mybir as mybir
    from concourse.tile import TileContext

    BF = mybir.dt.bfloat16
    F32 = mybir.dt.float32
    EXP = mybir.ActivationFunctionType.Exp

    nc = bass.Bass()

    xT = nc.declare_dram_parameter("xT", [DIM, N], BF, isOutput=False)
    metT = nc.declare_dram_parameter("metT", [DIM, N], BF, isOutput=False)
    yzT = nc.declare_dram_parameter("yzT", [DIM, N], BF, isOutput=False)
    wq = nc.declare_dram_parameter("wq", [DIM, NH * D], BF, isOutput=False)
    wk = nc.declare_dram_parameter("wk", [DIM, NH * D], BF, isOutput=False)
    wo = nc.declare_dram_parameter("wo", [NH * D, DIM], BF, isOutput=False)
    vin = nc.declare_dram_parameter("vin", [N, NH * (D + 1)], BF, isOutput=False)
    attnT = nc.declare_dram_parameter("attnT", [NH * N, N], BF, isOutput=True)
    out_part = nc.declare_dram_parameter("out_part", [2, N, DIM], F32, isOutput=True)

    # head h's transposed attn rows live at attnT[h*N + 128*j + p, i]
    attnT_v = attnT.rearrange("(h j p) i -> h j p i", h=NH, j=NJ)

    recip_dram = nc.dram_tensor("recip_scratch", [NH, N], BF)
    rowsum_dram = nc.dram_tensor("rowsum_scratch", [NH, N], F32)

    with TileContext(nc) as tc:
        with (
            tc.tile_pool(name="wpool", bufs=1) as wpool,
            tc.tile_pool(name="qk2", bufs=2 * NH) as qkpool,
            tc.tile_pool(name="vsb", bufs=1) as vpool,
            tc.tile_pool(name="small", bufs=2) as smpool,
            tc.tile_pool(name="rb", bufs=2) as rbpool,
            tc.tile_pool(name="ot", bufs=NH) as otpool,
            tc.tile_pool(name="osb", bufs=3) as opool,
            tc.tile_pool(name="psA", bufs=2, space="PSUM") as psA,
            tc.tile_pool(name="psO", bufs=1, space="PSUM") as psO,
        ):
            # ---- phase B: q/k projections ----
            # Q2T[h] = [q_h^T ; qm_h^T]  (128, N), d-major; K2T likewise.
            q2 = [qkpool.tile([128, N], BF, tag="q2", name=f"q2_{i}") for i in range(NH)]
            k2 = [qkpool.tile([128, N], BF, tag="q2", name=f"k2_{i}") for i in range(NH)]

            wq_sb = wpool.tile([128, DIM // 128, NH * D], BF, tag="wqk")
            nc.sync.dma_start(out=wq_sb, in_=wq.rearrange("(c p) m -> p c m", p=128))

            with tc.tile_pool(name="proj_in", bufs=3 * KC) as inpool:
                xs, ms, ys = [], [], []
                for k in range(KC):
                    tx = inpool.tile([128, N], BF, tag="pin", name=f"tx_{k}")
                    nc.sync.dma_start(out=tx, in_=xT[k * 128 : (k + 1) * 128, :])
                    tm = inpool.tile([128, N], BF, tag="pin", name=f"tm_{k}")
                    nc.sync.dma_start(out=tm, in_=metT[k * 128 : (k + 1) * 128, :])
                    xs.append(tx)
                    ms.append(tm)
                wk_sb = wpool.tile([128, DIM // 128, NH * D], BF, tag="wqk2")
                nc.sync.dma_start(
                    out=wk_sb, in_=wk.rearrange("(c p) m -> p c m", p=128)
                )
                for k in range(KC):
                    ty = inpool.tile([128, N], BF, tag="pin", name=f"ty_{k}")
                    nc.sync.dma_start(out=ty, in_=yzT[k * 128 : (k + 1) * 128, :])
                    ys.append(ty)
                # v (host pre-projected): (128, NJ, NH, 65); col 64 is 1.0 so
                # out^T's PSUM row 64 accumulates the softmax denominator
                vbig = vpool.tile([128, NJ, NH, D + 1], BF, tag="v", name="vbig")
                nc.sync.dma_start(
                    out=vbig,
                    in_=vin.rearrange("(j p) (h e) -> p j h e", p=128, h=NH),
                )
                v_sb = [vbig[:, j] for j in range(NJ)]
                wo_sb = []
                for h in range(NH):
                    t = wpool.tile([D, DIM], BF, tag=f"wo{h}", name=f"wo_sb_{h}")
                    nc.sync.dma_start(out=t, in_=wo[h * D : (h + 1) * D, :])
                    wo_sb.append(t)

                def project_pair(dst, w_sb, rhs_a, rhs_b, h):
                    # dst[0:64] = (a @ W_h)^T ; dst[64:128] = (b @ W_h)^T
                    for n in range(4):
                        ns = slice(n * 512, (n + 1) * 512)
                        ps = psA.tile([128, 1024], mybir.dt.float32, tag="ps", name="ps")
                        for k in range(KC):
                            nc.tensor.matmul(
                                out=ps[0:64, 0:512],
                                lhsT=w_sb[:, k, h * D : (h + 1) * D],
                                rhs=rhs_a[k][:, ns],
                                start=(k == 0),
                                stop=(k == KC - 1),
                            )
                        for k in range(KC):
                            nc.tensor.matmul(
                                out=ps[64:128, 512:1024],
                                lhsT=w_sb[:, k, h * D : (h + 1) * D],
                                rhs=rhs_b[k][:, ns],
                                start=(k == 0),
                                stop=(k == KC - 1),
                            )
                        nc.scalar.copy(out=dst[0:64, ns], in_=ps[0:64, 0:512])
                        nc.scalar.copy(
                            out=dst[64:128, ns], in_=ps[64:128, 512:1024]
                        )

                for h in range(NH):
                    project_pair(q2[h], wq_sb, xs, ms, h)
                for h in range(NH):
                    project_pair(k2[h], wk_sb, ys, ms, h)

            # ---- phase C: scores -> exp -> out^T, per head ----
            ot = []  # normalized out^T per head: (64, N) bf16
            with (
                tc.tile_pool(name="et", bufs=18) as etpool,
                tc.tile_pool(name="stage", bufs=2) as stpool,
            ):
                for h in range(NH):
                    outp = psO.tile([65, N], mybir.dt.float32, tag="po", name="outp")
                    ets = []
                    for j in range(NJ):
                        et = etpool.tile([128, N], BF, tag="et", name="et")
                        for half in range(2):
                            sp = psA.tile([128, 1024], mybir.dt.float32, tag="ps", name="sp")
                            for ii in range(2):
                                i0 = half * 1024 + ii * 512
                                nc.tensor.matmul(
                                    out=sp[:, ii * 512 : (ii + 1) * 512],
                                    lhsT=k2[h][:, j * 128 : (j + 1) * 128],
                                    rhs=q2[h][:, i0 : i0 + 512],
                                    start=True,
                                    stop=True,
                                )
                            nc.scalar.activation(
                                out=et[:, half * 1024 : (half + 1) * 1024],
                                in_=sp,
                                func=EXP,
                                scale=SCALE,
                            )
                        for ii in range(4):
                            nc.tensor.matmul(
                                out=outp[:, ii * 512 : (ii + 1) * 512],
                                lhsT=v_sb[j][:, h, :],
                                rhs=et[:, ii * 512 : (ii + 1) * 512],
                                start=(j == 0),
                                stop=(j == NJ - 1),
                                skip_group_check=True,
                            )
                        ets.append(et)

                    # epilogue: denominators -> reciprocal -> broadcast.
                    # The 2048 sums live on ONE partition (PSUM row 64); a
                    # 1-wide reciprocal is ~13us.  Bounce through DRAM to
                    # spread them over 128 partitions (FD=16) instead.
                    rb_seed = smpool.tile([65, N], mybir.dt.float32, tag="seed", name="rb_seed", bufs=1)
                    nc.scalar.copy(out=rb_seed[64:65, :], in_=outp[64:65, :])
                    nc.sync.dma_start(
                        out=rowsum_dram[h : h + 1, :], in_=rb_seed[64:65, :]
                    )
                    rs_sp = smpool.tile([128, N // 128], mybir.dt.float32, tag="rsp", name="rs_sp", bufs=2)
                    nc.sync.dma_start(
                        out=rs_sp,
                        in_=rowsum_dram[h].rearrange("(p c) -> p c", p=128),
                    )
                    rc_sp = smpool.tile([128, N // 128], BF, tag="rcp", name="rc_sp", bufs=2)
                    with nc.allow_low_precision(reason="bf16 softmax denom"):
                        nc.vector.reciprocal(out=rc_sp, in_=rs_sp)
                    nc.sync.dma_start(
                        out=recip_dram[h].rearrange("(p c) -> p c", p=128), in_=rc_sp
                    )
                    recipB = rbpool.tile([128, N], BF, tag="rb", name="recipB")
                    nc.sync.dma_start(
                        out=recipB,
                        in_=bass.AP(
                            tensor=recip_dram,
                            offset=h * N,
                            ap=[[0, 128], [1, N]],
                        ),
                    )
                    # normalized out^T for the Wo matmul
                    ot_raw = smpool.tile([D, N], BF, tag="otraw", name="ot_raw")
                    nc.scalar.copy(out=ot_raw, in_=outp[0:D, :])
                    ot_h = otpool.tile([D, N], BF, tag="ot", name=f"ot_{h}")
                    nc.vector.tensor_mul(ot_h, ot_raw, recipB[0:D, :])
                    ot.append(ot_h)

                    # normalized attn, staged 4 j-chunks per DMA (2 MiB)
                    for g in range(4):
                        stg = stpool.tile([128, 4, N], BF, tag="stg", name="stg")
                        for jj in range(4):
                            j = g * 4 + jj
                            nc.vector.tensor_mul(stg[:, jj, :], ets[j], recipB)
                        nc.sync.dma_start(
                            out=attnT_v[h, g * 4 : (g + 1) * 4].rearrange(
                                "j p i -> p j i"
                            ),
                            in_=stg,
                        )

                    # Wo partial for this head pair once both ot's exist;
                    # the pair-0 matmuls overlap heads 2-3, only pair-1
                    # trails the last head.  Host sums the two halves.
                    if h % 2 == 1:
                        pair = h // 2
                        for ic in range(N // 128):
                            ps = psA.tile(
                                [128, 1024], mybir.dt.float32, tag="ps", name="ps_wo"
                            )
                            for hh in (h - 1, h):
                                nc.tensor.matmul(
                                    out=ps[:, 0:512],
                                    lhsT=ot[hh][:, ic * 128 : (ic + 1) * 128],
                                    rhs=wo_sb[hh],
                                    start=(hh == h - 1),
                                    stop=(hh == h),
                                )
                            osb = opool.tile(
                                [128, DIM], mybir.dt.float32, tag="osb", name="osb"
                            )
                            nc.vector.tensor_copy(out=osb, in_=ps[:, 0:512])
                            nc.sync.dma_start(
                                out=out_part[pair, ic * 128 : (ic + 1) * 128, :],
                                in_=osb,
                            )



    return nc


def _get_nc():
    global _built
    if _built is None:
        _built = _build_nc()
    return _built


_legalized = False


def _ensure_legalized(nc):
    # applied only for the hardware path: CoreSim rejects the bare NoOps
    global _legalized
    if not _legalized:
        _legalize_mm_waits(nc)
        _legalized = True


_last_results = None


def _pack_v(yzb, Wv_sl):
    # v = yz[b] @ Wv_slice in bf16-rounded f32 (matches device compute
    # precision), reshaped (N, NH, D) with a ones column per head
    v = yzb.astype(BF16).astype(np.float32) @ Wv_sl.astype(BF16).astype(np.float32)
    v = v.reshape(N, NH, D)
    out = np.ones((N, NH, D + 1), dtype=BF16)
    out[:, :, 0:D] = v.astype(BF16)
    return out.reshape(N, NH * (D + 1))


def kernel(x, met, yz, Wq, Wk, Wv, Wo, bo):
    global _last_results
    from concourse.bass_utils import run_bass_kernel_spmd

    x = np.asarray(x, dtype=np.float32)
    met = np.asarray(met, dtype=np.float32)
    yz = np.asarray(yz, dtype=np.float32)
    Wq = np.asarray(Wq, dtype=np.float32)
    Wk = np.asarray(Wk, dtype=np.float32)
    Wv = np.asarray(Wv, dtype=np.float32)
    Wo = np.asarray(Wo, dtype=np.float32)
    bo = np.asarray(bo, dtype=np.float32)

    nc = _get_nc()
    _ensure_legalized(nc)

    in_maps = []
    for core in range(NCORES):
        b = core // 2
        hg = core % 2
        cs = slice(hg * NH * D, (hg + 1) * NH * D)
        in_maps.append(
            {
                "xT": np.ascontiguousarray(x[b].T).astype(BF16),
                "metT": np.ascontiguousarray(met[b].T).astype(BF16),
                "yzT": np.ascontiguousarray(yz[b].T).astype(BF16),
                "wq": Wq[:, cs].astype(BF16),
                "wk": Wk[:, cs].astype(BF16),
                "wo": np.ascontiguousarray(Wo[cs, :]).astype(BF16),
                "vin": _pack_v(yz[b], Wv[:, cs]),
            }
        )

    res = run_bass_kernel_spmd(
        nc,
        in_maps,
        core_ids=list(range(NCORES)),
        trace=bool(os.environ.get("BASS_TRACE")),
    )
    _last_results = res

    out = np.empty((B, N, DIM), dtype=np.float32)
    attn = np.empty((B * HEADS, N, N), dtype=np.float32)
    for core in range(NCORES):
        b = core // 2
        hg = core % 2
        r = res.results[core]
        at = np.asarray(r["attnT"]).reshape(NH, N, N)
        for h in range(NH):
            attn[b * HEADS + hg * NH + h] = at[h].T.astype(np.float32)
        op = np.asarray(r["out_part"])
        if hg == 0:
            out[b] = op[0] + op[1]
        else:
            out[b] += op[0] + op[1]
    out += bo[None, None, :]
    return out, attn


# revision 36
# speedup vs baseline: 1.0999x; 1.0999x over previous
"""Cross-attention (queries from x, keys/values from yz, additive met bias)
distributed over 8 TRN2 NeuronCores.

Module (B=4, N=2048, DIM=512, H=8, D=64):
    q  = split_heads(x @ Wq);  k = split_heads(yz @ Wk);  v = split_heads(yz @ Wv)
    sim  = (q k^T + q_met k_met^T) * D**-0.5          # (B*H, N, N)
    attn = softmax(sim, -1)
    out  = merge_heads(attn @ v) @ Wo + bo
    returns (out, attn)

Sharding: batch*head parallel.  Core i handles batch b=i//2, heads
[4*(i%2), 4*(i%2)+4).  Weights are sliced per head group; x/met/yz are
pre-transposed to DIM-major bf16 on host; v (+ a ones column) is host
pre-projected.  No collectives: the only cross-core reduction (the two
half-head Wo partials per batch) is a host-side add during unshard.

Device layout: everything j-major (S^T):
 * scores: S^T = K2T.T @ Q2T with the met bias packed along the K=128
   contraction ([k^T;km^T] x [q^T;qm^T]) -- the bias add is free.
 * out:    out^T = v(65-col,ones-augmented)-as-lhsT @ exp(S^T); PSUM row 64
   accumulates the softmax denominators (free row-sum).
 * attn is written transposed in bf16 (dense DMA rows) and un-transposed /
   upcast to f32 on host during unshard.

Walrus only allows ONE sync-wait on a Matmult; Tile sometimes emits 2-3
(recycled-PSUM WAR + WAW).  _legalize_mm_waits post-processes the BIR and
hoists every excess wait onto an inserted same-engine NoOp directly before
the matmul (the NX sequencer executes them in order -- semantically
identical).
"""

import os
import sys

import numpy as np

_TRN_REPO = "/opt/trn_rl_repo"
if _TRN_REPO not in sys.path:
    sys.path.insert(0, _TRN_REPO)

import ml_dtypes

BF16 = ml_dtypes.bfloat16

B = 4
N = 2048
DIM = 512
HEADS = 8
D = 64
INNER = HEADS * D
SCALE = float(D) ** -0.5  # 0.125
NCORES = 8
NH = 4  # heads per core
NJ = N // 128  # 16 j-chunks
KC = DIM // 128  # 4 contraction chunks for projections

_built = None


def _legalize_mm_waits(nc):
    """Walrus codegen accepts at most one sync-wait per Matmult.  Move any
    extra waits onto same-engine NoOps inserted right before the matmul."""
    import concourse.mybir as mybir

    cnt = 0
    for fn in nc.m.functions:
        for blk in fn.blocks:
            out = []
            changed = False
            for ins in blk.instructions:
                si = getattr(ins, "sync_info", None)
                hoistable = type(ins).__name__ in {
                    "InstMatmult",
                    "InstMatmultMx",
                    "InstActivation",
                    "InstLdweights",
                    "InstDMA",
                    "InstDMACopy",
                    "InstDmaTransposeAnt",
                    "InstTensorCopy",
                    "InstTensorTensor",
                    "InstTensorScalarPtr",
                    "InstTensorReduce",
                    "InstMemSet",
                    "InstDrain",
                    "InstReciprocal",
                    "InstCopy",
                    "InstIota",
                    "InstBNStats",
                    "InstBNStatsAggregate",
                }
                if (
                    hoistable
                    and si is not None
                    and si.on_wait
                    and len(si.on_wait) > 1
                ):
                    for wt in si.on_wait:
                        out.append(
                            mybir.InstNoOp(
                                name=f"I-mmwait-{cnt}",
                                engine=ins.engine,
                                ins=[],
                                outs=[],
                                sync_info=mybir.SyncInfo(
                                    on_wait=[wt], on_update=[]
                                ),
                            )
                        )
                        cnt += 1
                    ins.sync_info = mybir.SyncInfo(
                        on_wait=[], on_update=si.on_update
                    )
                    changed = True
                out.append(ins)
            if changed:
                blk.instructions[:] = out
    return cnt


def _build_nc():
    import concourse.bass as bass
    import concourse.mybir as mybir
    from concourse.tile import TileContext

    BF = mybir.dt.bfloat16
    F32 = mybir.dt.float32
    EXP = mybir.ActivationFunctionType.Exp

    nc = bass.Bass()

    xT = nc.declare_dram_parameter("xT", [DIM, N], BF, isOutput=False)
    metT = nc.declare_dram_parameter("metT", [DIM, N], BF, isOutput=False)
    yzT = nc.declare_dram_parameter("yzT", [DIM, N], BF, isOutput=False)
    wq = nc.declare_dram_parameter("wq", [DIM, NH * D], BF, isOutput=False)
    wk = nc.declare_dram_parameter("wk", [DIM, NH * D], BF, isOutput=False)
    wo = nc.declare_dram_parameter("wo", [NH * D, DIM], BF, isOutput=False)
    vin = nc.declare_dram_parameter("vin", [N, NH * (D + 1)], BF, isOutput=False)
    attnT = nc.declare_dram_parameter("attnT", [NH * N, N], BF, isOutput=True)
    out_part = nc.declare_dram_parameter("out_part", [N, DIM], F32, isOutput=True)

    # head h's transposed attn rows live at attnT[h*N + 128*j + p, i]
    attnT_v = attnT.rearrange("(h j p) i -> h j p i", h=NH, j=NJ)

    recip_dram = nc.dram_tensor("recip_scratch", [NH, N], BF)
    rowsum_dram = nc.dram_tensor("rowsum_scratch", [NH, N], F32)

    with TileContext(nc) as tc:
        with (
            tc.tile_pool(name="wpool", bufs=1) as wpool,
            tc.tile_pool(name="qk2", bufs=2 * NH) as qkpool,
            tc.tile_pool(name="vsb", bufs=1) as vpool,
            tc.tile_pool(name="small", bufs=2) as smpool,
            tc.tile_pool(name="rb", bufs=2) as rbpool,
            tc.tile_pool(name="ot", bufs=NH) as otpool,
            tc.tile_pool(name="osb", bufs=3) as opool,
            tc.tile_pool(name="psA", bufs=2, space="PSUM") as psA,
            tc.tile_pool(name="psO", bufs=1, space="PSUM") as psO,
        ):
            # ---- phase B: q/k projections ----
            # Q2T[h] = [q_h^T ; qm_h^T]  (128, N), d-major; K2T likewise.
            q2 = [qkpool.tile([128, N], BF, tag="q2", name=f"q2_{i}") for i in range(NH)]
            k2 = [qkpool.tile([128, N], BF, tag="q2", name=f"k2_{i}") for i in range(NH)]

            wq_sb = wpool.tile([128, DIM // 128, NH * D], BF, tag="wqk")
            nc.sync.dma_start(out=wq_sb, in_=wq.rearrange("(c p) m -> p c m", p=128))

            with tc.tile_pool(name="proj_in", bufs=3 * KC) as inpool:
                xs, ms, ys = [], [], []
                for k in range(KC):
                    tx = inpool.tile([128, N], BF, tag="pin", name=f"tx_{k}")
                    nc.sync.dma_start(out=tx, in_=xT[k * 128 : (k + 1) * 128, :])
                    tm = inpool.tile([128, N], BF, tag="pin", name=f"tm_{k}")
                    nc.sync.dma_start(out=tm, in_=metT[k * 128 : (k + 1) * 128, :])
                    xs.append(tx)
                    ms.append(tm)
                wk_sb = wpool.tile([128, DIM // 128, NH * D], BF, tag="wqk2")
                nc.sync.dma_start(
                    out=wk_sb, in_=wk.rearrange("(c p) m -> p c m", p=128)
                )
                for k in range(KC):
                    ty = inpool.tile([128, N], BF, tag="pin", name=f"ty_{k}")
                    nc.sync.dma_start(out=ty, in_=yzT[k * 128 : (k + 1) * 128, :])
                    ys.append(ty)
                # v (host pre-projected): (128, NJ, NH, 65); col 64 is 1.0 so
                # out^T's PSUM row 64 accumulates the softmax denominator
                vbig = vpool.tile([128, NJ, NH, D + 1], BF, tag="v", name="vbig")
                nc.sync.dma_start(
                    out=vbig,
                    in_=vin.rearrange("(j p) (h e) -> p j h e", p=128, h=NH),
                )
                v_sb = [vbig[:, j] for j in range(NJ)]
                wo_sb = []
                for h in range(NH):
                    t = wpool.tile([D, DIM], BF, tag=f"wo{h}", name=f"wo_sb_{h}")
                    nc.sync.dma_start(out=t, in_=wo[h * D : (h + 1) * D, :])
                    wo_sb.append(t)

                def project_pair(dst, w_sb, rhs_a, rhs_b, h):
                    # dst[0:64] = (a @ W_h)^T ; dst[64:128] = (b @ W_h)^T
                    for n in range(4):
                        ns = slice(n * 512, (n + 1) * 512)
                        ps = psA.tile([128, 1024], mybir.dt.float32, tag="ps", name="ps")
                        for k in range(KC):
                            nc.tensor.matmul(
                                out=ps[0:64, 0:512],
                                lhsT=w_sb[:, k, h * D : (h + 1) * D],
                                rhs=rhs_a[k][:, ns],
                                start=(k == 0),
                                stop=(k == KC - 1),
                            )
                        for k in range(KC):
                            nc.tensor.matmul(
                                out=ps[64:128, 512:1024],
                                lhsT=w_sb[:, k, h * D : (h + 1) * D],
                                rhs=rhs_b[k][:, ns],
                                start=(k == 0),
                                stop=(k == KC - 1),
                            )
                        nc.scalar.copy(out=dst[0:64, ns], in_=ps[0:64, 0:512])
                        nc.scalar.copy(
                            out=dst[64:128, ns], in_=ps[64:128, 512:1024]
                        )

                for h in range(NH):
                    project_pair(q2[h], wq_sb, xs, ms, h)
                for h in range(NH):
                    project_pair(k2[h], wk_sb, ys, ms, h)

            # ---- phase C: scores -> exp -> out^T, per head ----
            ot = []  # normalized out^T per head: (64, N) bf16
            with (
                tc.tile_pool(name="et", bufs=18) as etpool,
                tc.tile_pool(name="stage", bufs=2) as stpool,
            ):
                for h in range(NH):
                    outp = psO.tile([65, N], mybir.dt.float32, tag="po", name="outp")
                    ets = []
                    for j in range(NJ):
                        et = etpool.tile([128, N], BF, tag="et", name="et")
                        for half in range(2):
                            sp = psA.tile([128, 1024], mybir.dt.float32, tag="ps", name="sp")
                            for ii in range(2):
                                i0 = half * 1024 + ii * 512
                                nc.tensor.matmul(
                                    out=sp[:, ii * 512 : (ii + 1) * 512],
                                    lhsT=k2[h][:, j * 128 : (j + 1) * 128],
                                    rhs=q2[h][:, i0 : i0 + 512],
                                    start=True,
                                    stop=True,
                                )
                            nc.scalar.activation(
                                out=et[:, half * 1024 : (half + 1) * 1024],
                                in_=sp,
                                func=EXP,
                                scale=SCALE,
                            )
                        for ii in range(4):
                            nc.tensor.matmul(
                                out=outp[:, ii * 512 : (ii + 1) * 512],
                                lhsT=v_sb[j][:, h, :],
                                rhs=et[:, ii * 512 : (ii + 1) * 512],
                                start=(j == 0),
                                stop=(j == NJ - 1),
                                skip_group_check=True,
                            )
                        ets.append(et)

                    # epilogue: denominators -> reciprocal -> broadcast.
                    # The 2048 sums live on ONE partition (PSUM row 64); a
                    # 1-wide reciprocal is ~13us.  Bounce through DRAM to
                    # spread them over 128 partitions (FD=16) instead.
                    rb_seed = smpool.tile([65, N], mybir.dt.float32, tag="seed", name="rb_seed", bufs=1)
                    nc.scalar.copy(out=rb_seed[64:65, :], in_=outp[64:65, :])
                    nc.sync.dma_start(
                        out=rowsum_dram[h : h + 1, :], in_=rb_seed[64:65, :]
                    )
                    rs_sp = smpool.tile([128, N // 128], mybir.dt.float32, tag="rsp", name="rs_sp", bufs=2)
                    nc.sync.dma_start(
                        out=rs_sp,
                        in_=rowsum_dram[h].rearrange("(p c) -> p c", p=128),
                    )
                    rc_sp = smpool.tile([128, N // 128], BF, tag="rcp", name="rc_sp", bufs=2)
                    with nc.allow_low_precision(reason="bf16 softmax denom"):
                        nc.vector.reciprocal(out=rc_sp, in_=rs_sp)
                    nc.sync.dma_start(
                        out=recip_dram[h].rearrange("(p c) -> p c", p=128), in_=rc_sp
                    )
                    recipB = rbpool.tile([128, N], BF, tag="rb", name="recipB")
                    nc.sync.dma_start(
                        out=recipB,
                        in_=bass.AP(
                            tensor=recip_dram,
                            offset=h * N,
                            ap=[[0, 128], [1, N]],
                        ),
                    )
                    # normalized out^T for the Wo matmul
                    ot_raw = smpool.tile([D, N], BF, tag="otraw", name="ot_raw")
                    nc.scalar.copy(out=ot_raw, in_=outp[0:D, :])
                    ot_h = otpool.tile([D, N], BF, tag="ot", name=f"ot_{h}")
                    nc.vector.tensor_mul(ot_h, ot_raw, recipB[0:D, :])
                    ot.append(ot_h)

                    # normalized attn, staged 4 j-chunks per DMA (2 MiB)
                    for g in range(4):
                        stg = stpool.tile([128, 4, N], BF, tag="stg", name="stg")
                        for jj in range(4):
                            j = g * 4 + jj
                            nc.vector.tensor_mul(stg[:, jj, :], ets[j], recipB)
                        nc.sync.dma_start(
                            out=attnT_v[h, g * 4 : (g + 1) * 4].rearrange(
                                "j p i -> p j i"
                            ),
                            in_=stg,
                        )

            # ---- Wo partial: out_part[0] = concat_h(out_h) @ Wo_rows ----
            for ic in range(N // 128):
                ps = psA.tile([128, 1024], mybir.dt.float32, tag="ps", name="ps_wo")
                for hh in range(NH):
                    nc.tensor.matmul(
                        out=ps[:, 0:512],
                        lhsT=ot[hh][:, ic * 128 : (ic + 1) * 128],
                        rhs=wo_sb[hh],
                        start=(hh == 0),
                        stop=(hh == NH - 1),
                    )
                osb = opool.tile([128, DIM], mybir.dt.float32, tag="osb", name="osb")
                nc.vector.tensor_copy(out=osb, in_=ps[:, 0:512])
                nc.sync.dma_start(
                    out=out_part[ic * 128 : (ic + 1) * 128, :], in_=osb
                )




    return nc


def _get_nc():
    global _built
    if _built is None:
        _built = _build_nc()
    return _built


_legalized = False


def _ensure_legalized(nc):
    # applied only for the hardware path: CoreSim rejects the bare NoOps
    global _legalized
    if not _legalized:
        _legalize_mm_waits(nc)
        _legalized = True


_last_results = None


def _pack_v(yzb, Wv_sl):
    # v = yz[b] @ Wv_slice in bf16-rounded f32 (matches device compute
    # precision), reshaped (N, NH, D) with a ones column per head
    v = yzb.astype(BF16).astype(np.float32) @ Wv_sl.astype(BF16).astype(np.float32)
    v = v.reshape(N, NH, D)
    out = np.ones((N, NH, D + 1), dtype=BF16)
    out[:, :, 0:D] = v.astype(BF16)
    return out.reshape(N, NH * (D + 1))


def kernel(x, met, yz, Wq, Wk, Wv, Wo, bo):
    global _last_results
    from concourse.bass_utils import run_bass_kernel_spmd

    x = np.asarray(x, dtype=np.float32)
    met = np.asarray(met, dtype=np.float32)
    yz = np.asarray(yz, dtype=np.float32)
    Wq = np.asarray(Wq, dtype=np.float32)
    Wk = np.asarray(Wk, dtype=np.float32)
    Wv = np.asarray(Wv, dtype=np.float32)
    Wo = np.asarray(Wo, dtype=np.float32)
    bo = np.asarray(bo, dtype=np.float32)

    nc = _get_nc()
    _ensure_legalized(nc)

    in_maps = []
    for core in range(NCORES):
        b = core // 2
        hg = core % 2
        cs = slice(hg * NH * D, (hg + 1) * NH * D)
        in_maps.append(
            {
                "xT": np.ascontiguousarray(x[b].T).astype(BF16),
                "metT": np.ascontiguousarray(met[b].T).astype(BF16),
                "yzT": np.ascontiguousarray(yz[b].T).astype(BF16),
                "wq": Wq[:, cs].astype(BF16),
                "wk": Wk[:, cs].astype(BF16),
                "wo": np.ascontiguousarray(Wo[cs, :]).astype(BF16),
                "vin": _pack_v(yz[b], Wv[:, cs]),
            }
        )

    res = run_bass_kernel_spmd(
        nc,
        in_maps,
        core_ids=list(range(NCORES)),
        trace=bool(os.environ.get("BASS_TRACE")),
    )
    _last_results = res

    out = np.empty((B, N, DIM), dtype=np.float32)
    attn = np.empty((B * HEADS, N, N), dtype=np.float32)
    for core in range(NCORES):
        b = core // 2
        hg = core % 2
        r = res.results[core]
        at = np.asarray(r["attnT"]).reshape(NH, N, N)
        for h in range(NH):
            attn[b * HEADS + hg * NH + h] = at[h].T.astype(np.float32)
        if hg == 0:
            out[b] = np.asarray(r["out_part"])
        else:
            out[b] += np.asarray(r["out_part"])
    out += bo[None, None, :]
    return out, attn
